# revision 15
# baseline (speedup 1.0000x reference)
"""Trainium2 Bass kernel for nn_CausalSelfAttentionSynapse.

Math (per reference):
    qk = g @ W_lift.T                       # (B,T,2E)
    q,k heads of dim D=64; scores = q@k.T causal-masked
    lse[b,h,t] = logsumexp_{j<=t} scores[b,h,t,j]
    out[b,t]  = sum_h lse[b,h,t] * w[h],  w[h] = sum_g W_proj[g,h]

Sharding: 8 cores = 4 batches x 2 head-groups (8 heads each).
Per core (all on one NeuronCore):
  - PE-transpose g[b] and W_slice (head-group rows of W_lift) to get
    e-major layouts; lift matmuls (float32r, full PE rate) produce
    qT/kT (d on partitions) directly.
  - Per head pair, causal scores via 64x128 PE row tiling (2 heads
    run concurrently on the array), written to PSUM.
  - ScalarE exp (PSUM -> SBUF bf16), VectorE applies the triangular
    mask on the diagonal 128x128 block (multiplicative) and row-sums.
  - Ln + weighted head-sum on device; host adds the two head-group
    partials per batch.
"""

import numpy as np
import ml_dtypes

B, T, E, H = 4, 2048, 1024, 16
D = 64
NCORES = 8
NT = T // 128  # 16 g row tiles
NE = E // 128  # 8 e chunks
PAIRS = 4      # head pairs per core

_CACHE = {}


def _build():
    import concourse.bass as bass  # noqa: F401
    import concourse.tile as tile
    from concourse import bacc, mybir

    f32 = mybir.dt.float32
    f32r = mybir.dt.float32r
    bf16 = mybir.dt.bfloat16
    i32 = mybir.dt.int32
    EXP = mybir.ActivationFunctionType.Exp
    LN = mybir.ActivationFunctionType.Ln
    AX = mybir.AxisListType.X
    MUL = mybir.AluOpType.mult
    ADD = mybir.AluOpType.add
    SHR = mybir.AluOpType.logical_shift_right
    SUB = mybir.AluOpType.subtract
    AND = mybir.AluOpType.bitwise_and
    OR = mybir.AluOpType.bitwise_or
    LN2 = float(np.log(2.0))

    nc = bacc.Bacc("TRN2", target_bir_lowering=False, debug=False,
                   num_devices=NCORES)

    g_d = nc.dram_tensor("g_b", [T, E], f32, kind="ExternalInput").ap()
    w_d = nc.dram_tensor("w_slice", [E, E], f32, kind="ExternalInput").ap()
    wp_d = nc.dram_tensor("wp_cols", [16, 8], f32, kind="ExternalInput").ap()
    id_d = nc.dram_tensor("ident", [128, 128], f32, kind="ExternalInput").ap()
    tri_d = nc.dram_tensor("maskn", [128, 128], f32,
                           kind="ExternalInput").ap()
    out_d = nc.dram_tensor("out_part", [128, 16], f32,
                           kind="ExternalOutput").ap()

    with tile.TileContext(nc) as tc:
        with (
            tc.tile_pool(name="consts", bufs=1) as consts,
            tc.tile_pool(name="big", bufs=1) as big,
            tc.tile_pool(name="stage", bufs=3) as stage,
            tc.tile_pool(name="qkp", bufs=2) as qkp,
            tc.tile_pool(name="exps", bufs=3) as exps,
            tc.tile_pool(name="misc", bufs=1) as misc,
            tc.tile_pool(name="ps", bufs=4, space="PSUM") as ps,
        ):
            # ---- constants -------------------------------------------------
            ident = consts.tile([128, 128], f32, name="ident", tag="ident")
            nc.sync.dma_start(out=ident[:], in_=id_d[:])
            maskn = consts.tile([128, 128], f32, name="maskn",
                                tag="maskn")
            nc.sync.dma_start(out=maskn[:], in_=tri_d[:])
            wp = consts.tile([16, 8], f32, name="wp", tag="wp")
            nc.sync.dma_start(out=wp[:], in_=wp_d[:])
            ones16 = consts.tile([16, 128], f32, name="ones16", tag="ones16")
            nc.vector.memset(ones16[:], 1.0)
            heat = consts.tile([128, 128], bf16, name="heat", tag="heat")
            nc.vector.memset(heat[:], 0.0)

            # w[h] broadcast to all partitions: (128, 8)
            pw = ps.tile([128, 1024], f32, name="pw", tag="ps")
            nc.tensor.matmul(pw[:, 0:8], lhsT=ones16[:], rhs=wp[:],
                             start=True, stop=True)
            wb = consts.tile([128, 8], f32, name="wb", tag="wb")
            nc.vector.tensor_copy(wb[:], pw[:, 0:8])

            # ---- big SBUF layouts (float32r: rounded for PE full-rate) ----
            # gT[:, e*T + t]  : g transposed, e-chunk major
            gT = big.tile([128, NE * T], f32r, name="gT", tag="gT")
            # wT[:, e*E + f]  : W_slice transposed
            wT = big.tile([128, NE * E], f32r, name="wT", tag="wT")

            gT3 = gT.rearrange("p (e t) -> p e t", e=NE)
            wT3 = wT.rearrange("p (e f) -> p e f", e=NE)

            def g_transpose(ti):
                gst = stage.tile([128, E], f32, name=f"gst{ti}", tag="gst")
                nc.sync.dma_start(out=gst[:],
                                  in_=g_d[ti * 128:(ti + 1) * 128, :])
                pt = ps.tile([128, 1024], f32, name=f"ptg{ti}", tag="ps")
                for e in range(NE):
                    nc.tensor.transpose(pt[:, e * 128:(e + 1) * 128],
                                        gst[:, e * 128:(e + 1) * 128],
                                        ident[:])
                src = pt.rearrange("p (e t) -> p e t", e=NE)
                nc.vector.tensor_copy(
                    gT3[:, :, ti * 128:(ti + 1) * 128], src)

            def w_transpose(fi):
                wst = stage.tile([128, E], f32, name=f"wst{fi}", tag="wst")
                nc.sync.dma_start(out=wst[:],
                                  in_=w_d[fi * 128:(fi + 1) * 128, :])
                pt = ps.tile([128, 1024], f32, name=f"ptw{fi}", tag="ps")
                for e in range(NE):
                    nc.tensor.transpose(pt[:, e * 128:(e + 1) * 128],
                                        wst[:, e * 128:(e + 1) * 128],
                                        ident[:])
                src = pt.rearrange("p (e f) -> p e f", e=NE)
                nc.vector.tensor_copy(
                    wT3[:, :, fi * 128:(fi + 1) * 128], src)

            def lift_chunk(p, f, tcn, qkt):
                """Compute qkT for f-tile (2p+f), t-cols [tcn*1024, +1024)."""
                ft = 2 * p + f
                pt = ps.tile([128, 1024], f32, name=f"ptl{p}{f}{tcn}",
                             tag="ps")
                for e in range(NE):
                    lhsT = wT[:, e * E + ft * 128: e * E + ft * 128 + 128]
                    for half in range(2):
                        t0 = tcn * 1024 + half * 512
                        rhs = gT[:, e * T + t0: e * T + t0 + 512]
                        nc.tensor.matmul(
                            pt[:, half * 512:(half + 1) * 512],
                            lhsT=lhsT, rhs=rhs,
                            start=(e == 0), stop=(e == NE - 1))
                nc.vector.tensor_copy(
                    qkt[:, f * T + tcn * 1024: f * T + tcn * 1024 + 1024],
                    pt[:])

            sums_t = []
            for hh in range(8):
                st = misc.tile([128, 16], f32, name=f"sums{hh}", tag="sums",
                               bufs=8)
                sums_t.append(st)

            last_exp = [None]

            def scores_qtile(p, qi, qkt):
                """Causal scores + exp + fused mask/rowsum for both heads
                of pair p, query tile qi (rows qi*128..+128)."""
                kneed = 128 * (qi + 1)
                q0 = 128 * qi
                ntp = 1 if kneed <= 1024 else 2
                aqs = [exps.tile([128, 2], f32, name=f"aq{p}{qi}{h}",
                                 tag="aq", bufs=6) for h in range(2)]
                # per-j window: matmuls, psum diag mask, fused exp+rowsum
                for j in range(ntp):
                    lo = 1024 * j
                    cols = min(1024, kneed - lo)
                    pstj = [ps.tile([128, 1024], f32,
                                    name=f"pss{p}{qi}{h}{j}", tag="ps")
                            for h in range(2)]
                    ko = lo
                    while ko < lo + cols:
                        sz = min(512, lo + cols - ko)
                        for h in range(2):
                            lhsT = qkt[64 * h:64 * h + 64,
                                       qi * 128: qi * 128 + 128]
                            rhs = qkt[64 * h:64 * h + 64,
                                      T + ko: T + ko + sz]
                            nc.tensor.matmul(
                                pstj[h][:, ko - lo:ko - lo + sz],
                                lhsT=lhsT, rhs=rhs,
                                start=True, stop=True)
                        ko += sz
                    for h in range(2):
                        hh = 2 * p + h
                        if j == ntp - 1:
                            # additive causal mask on the diagonal block
                            dw = cols - 128
                            nc.vector.tensor_add(
                                pstj[h][:, dw:dw + 128],
                                pstj[h][:, dw:dw + 128], maskn[:])
                        scr = exps.tile([128, 1024], bf16,
                                        name=f"scr{p}{qi}{h}{j}", tag="scr")
                        acc_ap = (sums_t[hh][:, qi:qi + 1] if ntp == 1
                                  else aqs[h][:, j:j + 1])
                        nc.scalar.activation(
                            scr[:, 0:cols], pstj[h][:, 0:cols], EXP,
                            accum_out=acc_ap)
                        nc.tensor.ldweights(heat[:])
                        nc.tensor.ldweights(heat[:])
                if ntp == 2:
                    for h in range(2):
                        hh = 2 * p + h
                        nc.vector.tensor_add(
                            sums_t[hh][:, qi:qi + 1],
                            aqs[h][:, 0:1], aqs[h][:, 1:2])

            # ---- emission order (pipelining) ------------------------------
            for fi in range(2):
                w_transpose(fi)
            for ti in range(8):
                g_transpose(ti)

            qkts = {}
            qkts[0] = qkp.tile([128, 2 * T], f32r, name="qkt0", tag="qkt")
            for f in range(2):
                lift_chunk(0, f, 0, qkts[0])
            for ti in range(8, 16):
                g_transpose(ti)
            for f in range(2):
                lift_chunk(0, f, 1, qkts[0])
            for fi in range(2, 8):
                w_transpose(fi)

            for p in range(PAIRS):
                nxt = p + 1
                if nxt < PAIRS:
                    qkts[nxt] = qkp.tile([128, 2 * T], f32r,
                                         name=f"qkt{nxt}", tag="qkt")
                chunks = [(f, tcn) for tcn in range(2) for f in range(2)]
                ci = 0
                for qi in range(16):
                    scores_qtile(p, qi, qkts[p])
                    if nxt < PAIRS and qi % 4 == 3:
                        f, tcn = chunks[ci]
                        ci += 1
                        lift_chunk(nxt, f, tcn, qkts[nxt])

            # gate: forces the Ln chain to schedule after all exps are
            # done (keeps the ACT exp/ln table sets from thrashing)
            gate = misc.tile([128, 1], f32, name="gate", tag="gate")
            nc.vector.tensor_scalar(out=gate[:], in0=sums_t[7][:, 15:16],
                                    scalar1=0.0, scalar2=None, op0=MUL)

            # ---- finale: lse = ln(sums); out = sum_h w[h]*lse_h -----------
            # lse = ln(s) via exponent/mantissa split — ACT Ln is only
            # valid on ~[2^-66, 2^64] and sums span e^-49..e^56.
            # s = m * 2^(e-127), m in [1,2):
            #   lse = (e - 127)*ln2 + Ln(m)
            acc = [misc.tile([128, 16], f32, name=f"acc{i}", tag="acc",
                             bufs=2) for i in range(2)]
            nc.vector.memset(acc[0][:], 0.0)
            cur = 0
            for hh in range(8):
                u = sums_t[hh][:].bitcast(i32)
                ei = stage.tile([128, 16], i32, name=f"ei{hh}", tag="ei")
                nc.vector.tensor_scalar(out=ei[:], in0=u, scalar1=23,
                                        scalar2=None, op0=SHR)
                ef = stage.tile([128, 16], f32, name=f"ef{hh}", tag="ef")
                nc.vector.tensor_copy(ef[:], ei[:])
                nc.vector.tensor_scalar(out=ef[:], in0=ef[:], scalar1=127.0,
                                        scalar2=None, op0=SUB)
                mb = stage.tile([128, 16], i32, name=f"mb{hh}", tag="mb")
                nc.vector.tensor_scalar(out=mb[:], in0=u,
                                        scalar1=0x007FFFFF,
                                        scalar2=0x3F800000,
                                        op0=AND, op1=OR)
                lnm = stage.tile([128, 16], f32, name=f"lnm{hh}", tag="lnm")
                nc.scalar.activation(lnm[:], mb[:].bitcast(f32), LN,
                                     bias=gate[:, 0:1])
                lse = stage.tile([128, 16], f32, name=f"lse{hh}", tag="lse")
                nc.vector.scalar_tensor_tensor(
                    out=lse[:], in0=ef[:], scalar=LN2, in1=lnm[:],
                    op0=MUL, op1=ADD)
                nxt = 1 - cur
                nc.vector.scalar_tensor_tensor(
                    out=acc[nxt][:], in0=lse[:], scalar=wb[:, hh:hh + 1],
                    in1=acc[cur][:], op0=MUL, op1=ADD)
                cur = nxt
            nc.sync.dma_start(out=out_d[:], in_=acc[cur][:])

    nc.compile()
    return nc


def _get_nc():
    if "nc" not in _CACHE:
        _CACHE["nc"] = _build()
    return _CACHE["nc"]


def kernel(g, W_lift, W_proj):
    from concourse.bass_utils import run_bass_kernel_spmd

    g = np.asarray(g, dtype=np.float32)
    W_lift = np.asarray(W_lift, dtype=np.float32)
    W_proj = np.asarray(W_proj, dtype=np.float32)

    nc = _get_nc()
    ident = np.eye(128, dtype=np.float32)
    maskn = (1.0 - np.tril(np.ones((128, 128), dtype=np.float32))) * -1e30

    in_maps = []
    for core in range(NCORES):
        b, hg = core // 2, core % 2
        rows = []
        for p in range(PAIRS):
            h0 = hg * 8 + 2 * p
            h1 = h0 + 1
            rows += list(range(h0 * D, h0 * D + D))
            rows += list(range(h1 * D, h1 * D + D))
            rows += list(range(E + h0 * D, E + h0 * D + D))
            rows += list(range(E + h1 * D, E + h1 * D + D))
        w_slice = np.ascontiguousarray(W_lift[rows, :])
        in_maps.append({
            "g_b": np.ascontiguousarray(g[b]),
            "w_slice": w_slice,
            "wp_cols": np.ascontiguousarray(W_proj[:, hg * 8:hg * 8 + 8]),
            "ident": ident,
            "maskn": maskn,
        })

    res = run_bass_kernel_spmd(nc, in_maps, core_ids=list(range(NCORES)))
    _CACHE["last_results"] = res
    _CACHE["last_in_maps"] = in_maps

    out = np.zeros((B, T), dtype=np.float32)
    for core in range(NCORES):
        b = core // 2
        part = res.results[core]["out_part"]  # (128, 16)
        out[b] += part.T.reshape(-1)
    return out


# revision 17
# speedup vs baseline: 1.1338x; 1.1338x over previous
"""Trainium2 Bass kernel for nn_CausalSelfAttentionSynapse.

Math (per reference):
    qk = g @ W_lift.T                       # (B,T,2E)
    q,k heads of dim D=64; scores = q@k.T causal-masked
    lse[b,h,t] = logsumexp_{j<=t} scores[b,h,t,j]
    out[b,t]  = sum_h lse[b,h,t] * w[h],  w[h] = sum_g W_proj[g,h]

Sharding: 8 cores = 4 batches x 2 head-groups (8 heads each).
Per core (all on one NeuronCore):
  - PE-transpose g[b] and W_slice (head-group rows of W_lift) to get
    e-major layouts; lift matmuls (float32r, full PE rate) produce
    qT/kT (d on partitions) directly.
  - Per head pair, causal scores via 64x128 PE row tiling (2 heads
    run concurrently on the array), written to PSUM.
  - ScalarE exp (PSUM -> SBUF bf16), VectorE applies the triangular
    mask on the diagonal 128x128 block (multiplicative) and row-sums.
  - Ln + weighted head-sum on device; host adds the two head-group
    partials per batch.
"""

import numpy as np
import ml_dtypes

B, T, E, H = 4, 2048, 1024, 16
D = 64
NCORES = 8
NT = T // 128  # 16 g row tiles
NE = E // 128  # 8 e chunks
PAIRS = 4      # head pairs per core

_CACHE = {}


def _build():
    import concourse.bass as bass  # noqa: F401
    import concourse.tile as tile
    from concourse import bacc, mybir

    f32 = mybir.dt.float32
    f32r = mybir.dt.float32r
    bf16 = mybir.dt.bfloat16
    i32 = mybir.dt.int32
    EXP = mybir.ActivationFunctionType.Exp
    LN = mybir.ActivationFunctionType.Ln
    AX = mybir.AxisListType.X
    MUL = mybir.AluOpType.mult
    ADD = mybir.AluOpType.add
    SHR = mybir.AluOpType.logical_shift_right
    SUB = mybir.AluOpType.subtract
    AND = mybir.AluOpType.bitwise_and
    OR = mybir.AluOpType.bitwise_or
    LN2 = float(np.log(2.0))

    nc = bacc.Bacc("TRN2", target_bir_lowering=False, debug=False,
                   num_devices=NCORES)

    g_d = nc.dram_tensor("g_b", [T, E], f32, kind="ExternalInput").ap()
    w_d = nc.dram_tensor("w_slice", [E, E], f32, kind="ExternalInput").ap()
    wp_d = nc.dram_tensor("wp_cols", [16, 8], f32, kind="ExternalInput").ap()
    id_d = nc.dram_tensor("ident", [128, 128], f32, kind="ExternalInput").ap()
    tri_d = nc.dram_tensor("tri", [128, 128], bf16,
                           kind="ExternalInput").ap()
    out_d = nc.dram_tensor("out_part", [128, 16], f32,
                           kind="ExternalOutput").ap()

    with tile.TileContext(nc) as tc:
        with (
            tc.tile_pool(name="consts", bufs=1) as consts,
            tc.tile_pool(name="big", bufs=1) as big,
            tc.tile_pool(name="stage", bufs=3) as stage,
            tc.tile_pool(name="qkp", bufs=2) as qkp,
            tc.tile_pool(name="exps", bufs=3) as exps,
            tc.tile_pool(name="misc", bufs=1) as misc,
            tc.tile_pool(name="ps", bufs=4, space="PSUM") as ps,
        ):
            # ---- constants -------------------------------------------------
            ident = consts.tile([128, 128], f32, name="ident", tag="ident")
            nc.sync.dma_start(out=ident[:], in_=id_d[:])
            tri = consts.tile([128, 128], bf16, name="tri", tag="tri")
            nc.sync.dma_start(out=tri[:], in_=tri_d[:])
            wp = consts.tile([16, 8], f32, name="wp", tag="wp")
            nc.sync.dma_start(out=wp[:], in_=wp_d[:])
            ones16 = consts.tile([16, 128], f32, name="ones16", tag="ones16")
            nc.vector.memset(ones16[:], 1.0)

            # w[h] broadcast to all partitions: (128, 8)
            pw = ps.tile([128, 1024], f32, name="pw", tag="ps")
            nc.tensor.matmul(pw[:, 0:8], lhsT=ones16[:], rhs=wp[:],
                             start=True, stop=True)
            wb = consts.tile([128, 8], f32, name="wb", tag="wb")
            nc.vector.tensor_copy(wb[:], pw[:, 0:8])

            # ---- big SBUF layouts (float32r: rounded for PE full-rate) ----
            # gT[:, e*T + t]  : g transposed, e-chunk major
            gT = big.tile([128, NE * T], f32r, name="gT", tag="gT")
            # wT[:, e*E + f]  : W_slice transposed
            wT = big.tile([128, NE * E], f32r, name="wT", tag="wT")

            gT3 = gT.rearrange("p (e t) -> p e t", e=NE)
            wT3 = wT.rearrange("p (e f) -> p e f", e=NE)

            def g_transpose(ti):
                gst = stage.tile([128, E], f32, name=f"gst{ti}", tag="gst")
                nc.sync.dma_start(out=gst[:],
                                  in_=g_d[ti * 128:(ti + 1) * 128, :])
                pt = ps.tile([128, 1024], f32, name=f"ptg{ti}", tag="ps")
                for e in range(NE):
                    nc.tensor.transpose(pt[:, e * 128:(e + 1) * 128],
                                        gst[:, e * 128:(e + 1) * 128],
                                        ident[:])
                src = pt.rearrange("p (e t) -> p e t", e=NE)
                nc.vector.tensor_copy(
                    gT3[:, :, ti * 128:(ti + 1) * 128], src)

            def w_transpose(fi):
                wst = stage.tile([128, E], f32, name=f"wst{fi}", tag="wst")
                nc.sync.dma_start(out=wst[:],
                                  in_=w_d[fi * 128:(fi + 1) * 128, :])
                pt = ps.tile([128, 1024], f32, name=f"ptw{fi}", tag="ps")
                for e in range(NE):
                    nc.tensor.transpose(pt[:, e * 128:(e + 1) * 128],
                                        wst[:, e * 128:(e + 1) * 128],
                                        ident[:])
                src = pt.rearrange("p (e f) -> p e f", e=NE)
                nc.vector.tensor_copy(
                    wT3[:, :, fi * 128:(fi + 1) * 128], src)

            def lift_chunk(p, f, tcn, qkt):
                """Compute qkT for f-tile (2p+f), t-cols [tcn*1024, +1024)."""
                ft = 2 * p + f
                pt = ps.tile([128, 1024], f32, name=f"ptl{p}{f}{tcn}",
                             tag="ps")
                for e in range(NE):
                    lhsT = wT[:, e * E + ft * 128: e * E + ft * 128 + 128]
                    for half in range(2):
                        t0 = tcn * 1024 + half * 512
                        rhs = gT[:, e * T + t0: e * T + t0 + 512]
                        nc.tensor.matmul(
                            pt[:, half * 512:(half + 1) * 512],
                            lhsT=lhsT, rhs=rhs,
                            start=(e == 0), stop=(e == NE - 1))
                nc.vector.tensor_copy(
                    qkt[:, f * T + tcn * 1024: f * T + tcn * 1024 + 1024],
                    pt[:])

            sums_t = []
            for hh in range(8):
                st = misc.tile([128, 16], f32, name=f"sums{hh}", tag="sums",
                               bufs=8)
                sums_t.append(st)

            last_exp = [None]

            def scores_qtile(p, qi, qkt):
                """Causal scores + exp + fused mask/rowsum for both heads
                of pair p, query tile qi (rows qi*128..+128)."""
                kneed = 128 * (qi + 1)
                q0 = 128 * qi
                ntp = 1 if kneed <= 1024 else 2
                pst = [[ps.tile([128, 1024], f32,
                                name=f"pss{p}{qi}{h}{j}", tag="ps")
                        for j in range(ntp)] for h in range(2)]
                ko = 0
                while ko < kneed:
                    sz = min(512, kneed - ko)
                    j, off = ko // 1024, ko % 1024
                    for h in range(2):
                        lhsT = qkt[64 * h:64 * h + 64,
                                   qi * 128: qi * 128 + 128]
                        rhs = qkt[64 * h:64 * h + 64, T + ko: T + ko + sz]
                        nc.tensor.matmul(
                            pst[h][j][:, off:off + sz],
                            lhsT=lhsT, rhs=rhs,
                            start=True, stop=True)
                    ko += sz
                for h in range(2):
                    hh = 2 * p + h
                    eb = exps.tile([128, 2048], bf16,
                                   name=f"ebh{p}{qi}{h}", tag="eb")
                    for j in range(ntp):
                        cols = min(1024, kneed - 1024 * j)
                        nc.scalar.activation(
                            eb[:, 1024 * j:1024 * j + cols],
                            pst[h][j][:, 0:cols], EXP)
                    nc.vector.tensor_mul(
                        eb[:, kneed - 128:kneed],
                        eb[:, kneed - 128:kneed], tri[:])
                    nc.vector.reduce_sum(
                        out=sums_t[hh][:, qi:qi + 1],
                        in_=eb[:, 0:kneed], axis=AX)

            for fi in range(2):
                w_transpose(fi)
            for ti in range(8):
                g_transpose(ti)

            qkts = {}
            qkts[0] = qkp.tile([128, 2 * T], f32r, name="qkt0", tag="qkt")
            for f in range(2):
                lift_chunk(0, f, 0, qkts[0])
            for ti in range(8, 16):
                g_transpose(ti)
            for f in range(2):
                lift_chunk(0, f, 1, qkts[0])
            for fi in range(2, 8):
                w_transpose(fi)

            for qi in range(16):
                scores_qtile(0, qi, qkts[0])

            for p in range(1, PAIRS):
                qkts[p] = qkp.tile([128, 2 * T], f32r, name=f"qkt{p}",
                                   tag="qkt")
                for tcn in range(2):
                    for f in range(2):
                        lift_chunk(p, f, tcn, qkts[p])
                for qi in range(16):
                    scores_qtile(p, qi, qkts[p])

            # gate: forces the Ln chain to schedule after all exps are
            # done (keeps the ACT exp/ln table sets from thrashing)
            gate = misc.tile([128, 1], f32, name="gate", tag="gate")
            nc.vector.tensor_scalar(out=gate[:], in0=sums_t[7][:, 15:16],
                                    scalar1=0.0, scalar2=None, op0=MUL)

            # ---- finale: lse = ln(sums); out = sum_h w[h]*lse_h -----------
            # lse = ln(s) via exponent/mantissa split — ACT Ln is only
            # valid on ~[2^-66, 2^64] and sums span e^-49..e^56.
            # s = m * 2^(e-127), m in [1,2):
            #   lse = (e - 127)*ln2 + Ln(m)
            acc = [misc.tile([128, 16], f32, name=f"acc{i}", tag="acc",
                             bufs=2) for i in range(2)]
            nc.vector.memset(acc[0][:], 0.0)
            cur = 0
            for hh in range(8):
                u = sums_t[hh][:].bitcast(i32)
                ei = stage.tile([128, 16], i32, name=f"ei{hh}", tag="ei")
                nc.vector.tensor_scalar(out=ei[:], in0=u, scalar1=23,
                                        scalar2=None, op0=SHR)
                ef = stage.tile([128, 16], f32, name=f"ef{hh}", tag="ef")
                nc.vector.tensor_copy(ef[:], ei[:])
                nc.vector.tensor_scalar(out=ef[:], in0=ef[:], scalar1=127.0,
                                        scalar2=None, op0=SUB)
                mb = stage.tile([128, 16], i32, name=f"mb{hh}", tag="mb")
                nc.vector.tensor_scalar(out=mb[:], in0=u,
                                        scalar1=0x007FFFFF,
                                        scalar2=0x3F800000,
                                        op0=AND, op1=OR)
                lnm = stage.tile([128, 16], f32, name=f"lnm{hh}", tag="lnm")
                nc.scalar.activation(lnm[:], mb[:].bitcast(f32), LN,
                                     bias=gate[:, 0:1])
                lse = stage.tile([128, 16], f32, name=f"lse{hh}", tag="lse")
                nc.vector.scalar_tensor_tensor(
                    out=lse[:], in0=ef[:], scalar=LN2, in1=lnm[:],
                    op0=MUL, op1=ADD)
                nxt = 1 - cur
                nc.vector.scalar_tensor_tensor(
                    out=acc[nxt][:], in0=lse[:], scalar=wb[:, hh:hh + 1],
                    in1=acc[cur][:], op0=MUL, op1=ADD)
                cur = nxt
            nc.sync.dma_start(out=out_d[:], in_=acc[cur][:])

    nc.compile()
    return nc


def _get_nc():
    if "nc" not in _CACHE:
        _CACHE["nc"] = _build()
    return _CACHE["nc"]


def kernel(g, W_lift, W_proj):
    from concourse.bass_utils import run_bass_kernel_spmd

    g = np.asarray(g, dtype=np.float32)
    W_lift = np.asarray(W_lift, dtype=np.float32)
    W_proj = np.asarray(W_proj, dtype=np.float32)

    nc = _get_nc()
    ident = np.eye(128, dtype=np.float32)
    tri = np.tril(np.ones((128, 128), dtype=np.float32)).astype(
        ml_dtypes.bfloat16)

    in_maps = []
    for core in range(NCORES):
        b, hg = core // 2, core % 2
        rows = []
        for p in range(PAIRS):
            h0 = hg * 8 + 2 * p
            h1 = h0 + 1
            rows += list(range(h0 * D, h0 * D + D))
            rows += list(range(h1 * D, h1 * D + D))
            rows += list(range(E + h0 * D, E + h0 * D + D))
            rows += list(range(E + h1 * D, E + h1 * D + D))
        w_slice = np.ascontiguousarray(W_lift[rows, :])
        in_maps.append({
            "g_b": np.ascontiguousarray(g[b]),
            "w_slice": w_slice,
            "wp_cols": np.ascontiguousarray(W_proj[:, hg * 8:hg * 8 + 8]),
            "ident": ident,
            "tri": tri,
        })

    res = run_bass_kernel_spmd(nc, in_maps, core_ids=list(range(NCORES)))
    _CACHE["last_results"] = res
    _CACHE["last_in_maps"] = in_maps

    out = np.zeros((B, T), dtype=np.float32)
    for core in range(NCORES):
        b = core // 2
        part = res.results[core]["out_part"]  # (128, 16)
        out[b] += part.T.reshape(-1)
    return out


# revision 18
# speedup vs baseline: 1.1557x; 1.0194x over previous
"""Trainium2 Bass kernel for nn_CausalSelfAttentionSynapse.

Math (per reference):
    qk = g @ W_lift.T                       # (B,T,2E)
    q,k heads of dim D=64; scores = q@k.T causal-masked
    lse[b,h,t] = logsumexp_{j<=t} scores[b,h,t,j]
    out[b,t]  = sum_h lse[b,h,t] * w[h],  w[h] = sum_g W_proj[g,h]

Sharding: 8 cores = 4 batches x 2 head-groups (8 heads each).
Per core (all on one NeuronCore):
  - PE-transpose g[b] and W_slice (head-group rows of W_lift) to get
    e-major layouts; lift matmuls (float32r, full PE rate) produce
    qT/kT (d on partitions) directly.
  - Per head pair, causal scores via 64x128 PE row tiling (2 heads
    run concurrently on the array), written to PSUM.
  - ScalarE exp (PSUM -> SBUF bf16), VectorE applies the triangular
    mask on the diagonal 128x128 block (multiplicative) and row-sums.
  - Ln + weighted head-sum on device; host adds the two head-group
    partials per batch.
"""

import numpy as np
import ml_dtypes

B, T, E, H = 4, 2048, 1024, 16
D = 64
NCORES = 8
NT = T // 128  # 16 g row tiles
NE = E // 128  # 8 e chunks
PAIRS = 4      # head pairs per core

_CACHE = {}


def _build():
    import concourse.bass as bass  # noqa: F401
    import concourse.tile as tile
    from concourse import bacc, mybir

    f32 = mybir.dt.float32
    f32r = mybir.dt.float32r
    bf16 = mybir.dt.bfloat16
    i32 = mybir.dt.int32
    EXP = mybir.ActivationFunctionType.Exp
    LN = mybir.ActivationFunctionType.Ln
    AX = mybir.AxisListType.X
    MUL = mybir.AluOpType.mult
    ADD = mybir.AluOpType.add
    SHR = mybir.AluOpType.logical_shift_right
    SUB = mybir.AluOpType.subtract
    AND = mybir.AluOpType.bitwise_and
    OR = mybir.AluOpType.bitwise_or
    LN2 = float(np.log(2.0))

    nc = bacc.Bacc("TRN2", target_bir_lowering=False, debug=False,
                   num_devices=NCORES)

    g_d = nc.dram_tensor("g_b", [T, E], f32, kind="ExternalInput").ap()
    w_d = nc.dram_tensor("w_slice", [E, E], f32, kind="ExternalInput").ap()
    wp_d = nc.dram_tensor("wp_cols", [16, 8], f32, kind="ExternalInput").ap()
    id_d = nc.dram_tensor("ident", [128, 128], f32, kind="ExternalInput").ap()
    tri_d = nc.dram_tensor("tri", [128, 128], bf16,
                           kind="ExternalInput").ap()
    out_d = nc.dram_tensor("out_part", [128, 16], f32,
                           kind="ExternalOutput").ap()

    with tile.TileContext(nc) as tc:
        with (
            tc.tile_pool(name="consts", bufs=1) as consts,
            tc.tile_pool(name="big", bufs=1) as big,
            tc.tile_pool(name="stage", bufs=3) as stage,
            tc.tile_pool(name="qkp", bufs=2) as qkp,
            tc.tile_pool(name="exps", bufs=3) as exps,
            tc.tile_pool(name="misc", bufs=1) as misc,
            tc.tile_pool(name="ps", bufs=4, space="PSUM") as ps,
        ):
            # ---- constants -------------------------------------------------
            ident = consts.tile([128, 128], f32, name="ident", tag="ident")
            nc.sync.dma_start(out=ident[:], in_=id_d[:])
            tri = consts.tile([128, 128], bf16, name="tri", tag="tri")
            nc.sync.dma_start(out=tri[:], in_=tri_d[:])
            wp = consts.tile([16, 8], f32, name="wp", tag="wp")
            nc.sync.dma_start(out=wp[:], in_=wp_d[:])
            ones16 = consts.tile([16, 128], f32, name="ones16", tag="ones16")
            nc.vector.memset(ones16[:], 1.0)

            # w[h] broadcast to all partitions: (128, 8)
            pw = ps.tile([128, 1024], f32, name="pw", tag="ps")
            nc.tensor.matmul(pw[:, 0:8], lhsT=ones16[:], rhs=wp[:],
                             start=True, stop=True)
            wb = consts.tile([128, 8], f32, name="wb", tag="wb")
            nc.vector.tensor_copy(wb[:], pw[:, 0:8])

            # ---- big SBUF layouts (float32r: rounded for PE full-rate) ----
            # gT[:, e*T + t]  : g transposed, e-chunk major
            gT = big.tile([128, NE * T], f32r, name="gT", tag="gT")
            # wT[:, e*E + f]  : W_slice transposed
            wT = big.tile([128, NE * E], f32r, name="wT", tag="wT")

            gT3 = gT.rearrange("p (e t) -> p e t", e=NE)
            wT3 = wT.rearrange("p (e f) -> p e f", e=NE)

            def g_transpose(ti):
                gst = stage.tile([128, E], f32, name=f"gst{ti}", tag="gst")
                nc.sync.dma_start(out=gst[:],
                                  in_=g_d[ti * 128:(ti + 1) * 128, :])
                pt = ps.tile([128, 1024], f32, name=f"ptg{ti}", tag="ps")
                for e in range(NE):
                    nc.tensor.transpose(pt[:, e * 128:(e + 1) * 128],
                                        gst[:, e * 128:(e + 1) * 128],
                                        ident[:])
                src = pt.rearrange("p (e t) -> p e t", e=NE)
                nc.vector.tensor_copy(
                    gT3[:, :, ti * 128:(ti + 1) * 128], src)

            def w_transpose(fi):
                wst = stage.tile([128, E], f32, name=f"wst{fi}", tag="wst")
                nc.sync.dma_start(out=wst[:],
                                  in_=w_d[fi * 128:(fi + 1) * 128, :])
                pt = ps.tile([128, 1024], f32, name=f"ptw{fi}", tag="ps")
                for e in range(NE):
                    nc.tensor.transpose(pt[:, e * 128:(e + 1) * 128],
                                        wst[:, e * 128:(e + 1) * 128],
                                        ident[:])
                src = pt.rearrange("p (e f) -> p e f", e=NE)
                nc.vector.tensor_copy(
                    wT3[:, :, fi * 128:(fi + 1) * 128], src)

            def lift_chunk(p, f, tcn, qkt):
                """Compute qkT for f-tile (2p+f), t-cols [tcn*1024, +1024)."""
                ft = 2 * p + f
                pt = ps.tile([128, 1024], f32, name=f"ptl{p}{f}{tcn}",
                             tag="ps")
                for e in range(NE):
                    lhsT = wT[:, e * E + ft * 128: e * E + ft * 128 + 128]
                    for half in range(2):
                        t0 = tcn * 1024 + half * 512
                        rhs = gT[:, e * T + t0: e * T + t0 + 512]
                        nc.tensor.matmul(
                            pt[:, half * 512:(half + 1) * 512],
                            lhsT=lhsT, rhs=rhs,
                            start=(e == 0), stop=(e == NE - 1))
                nc.vector.tensor_copy(
                    qkt[:, f * T + tcn * 1024: f * T + tcn * 1024 + 1024],
                    pt[:])

            sums_t = []
            for hh in range(8):
                st = misc.tile([128, 16], f32, name=f"sums{hh}", tag="sums",
                               bufs=8)
                sums_t.append(st)

            last_exp = [None]

            def scores_qtile(p, qi, qkt):
                """Causal scores + exp + fused mask/rowsum for both heads
                of pair p, query tile qi (rows qi*128..+128)."""
                kneed = 128 * (qi + 1)
                q0 = 128 * qi
                ntp = 1 if kneed <= 1024 else 2
                pst = [[ps.tile([128, 1024], f32,
                                name=f"pss{p}{qi}{h}{j}", tag="ps")
                        for j in range(ntp)] for h in range(2)]
                ko = 0
                while ko < kneed:
                    sz = min(512, kneed - ko)
                    j, off = ko // 1024, ko % 1024
                    for h in range(2):
                        lhsT = qkt[64 * h:64 * h + 64,
                                   qi * 128: qi * 128 + 128]
                        rhs = qkt[64 * h:64 * h + 64, T + ko: T + ko + sz]
                        nc.tensor.matmul(
                            pst[h][j][:, off:off + sz],
                            lhsT=lhsT, rhs=rhs,
                            start=True, stop=True)
                    ko += sz
                for h in range(2):
                    hh = 2 * p + h
                    eb = exps.tile([128, 2048], bf16,
                                   name=f"ebh{p}{qi}{h}", tag="eb")
                    for j in range(ntp):
                        cols = min(1024, kneed - 1024 * j)
                        nc.scalar.activation(
                            eb[:, 1024 * j:1024 * j + cols],
                            pst[h][j][:, 0:cols], EXP)
                    nc.vector.tensor_mul(
                        eb[:, kneed - 128:kneed],
                        eb[:, kneed - 128:kneed], tri[:])
                    nc.vector.reduce_sum(
                        out=sums_t[hh][:, qi:qi + 1],
                        in_=eb[:, 0:kneed], axis=AX)

            for fi in range(2):
                w_transpose(fi)
            for ti in range(8):
                g_transpose(ti)

            qkts = {}
            qkts[0] = qkp.tile([128, 2 * T], f32r, name="qkt0", tag="qkt")
            for f in range(2):
                lift_chunk(0, f, 0, qkts[0])
            for ti in range(8, 16):
                g_transpose(ti)
            # scores qi<8 only need keys 0:1024 (lift tc0) — start the
            # exp pipeline while the rest of the lift streams in
            for qi in range(4):
                scores_qtile(0, qi, qkts[0])
            for f in range(2):
                lift_chunk(0, f, 1, qkts[0])
            for qi in range(4, 8):
                scores_qtile(0, qi, qkts[0])
            for fi in range(2, 8):
                w_transpose(fi)
            for qi in range(8, 16):
                scores_qtile(0, qi, qkts[0])

            # pairs 1-3: interleave the NEXT pair's lift chunks between
            # qtiles so ACT/DVE never drain during lift phases
            qkts[1] = qkp.tile([128, 2 * T], f32r, name="qkt1", tag="qkt")
            for tcn in range(2):
                for f in range(2):
                    lift_chunk(1, f, tcn, qkts[1])
            for p in range(1, PAIRS):
                nxt = p + 1
                if nxt < PAIRS:
                    qkts[nxt] = qkp.tile([128, 2 * T], f32r,
                                         name=f"qkt{nxt}", tag="qkt")
                chunks = [(f, tcn) for tcn in range(2) for f in range(2)]
                ci = 0
                for qi in range(16):
                    scores_qtile(p, qi, qkts[p])
                    if nxt < PAIRS and qi % 4 == 3:
                        f, tcn = chunks[ci]
                        ci += 1
                        lift_chunk(nxt, f, tcn, qkts[nxt])

            # gate: forces the Ln chain to schedule after all exps are
            # done (keeps the ACT exp/ln table sets from thrashing)
            gate = misc.tile([128, 1], f32, name="gate", tag="gate")
            nc.vector.tensor_scalar(out=gate[:], in0=sums_t[7][:, 15:16],
                                    scalar1=0.0, scalar2=None, op0=MUL)

            # ---- finale: lse = ln(sums); out = sum_h w[h]*lse_h -----------
            # lse = ln(s) via exponent/mantissa split — ACT Ln is only
            # valid on ~[2^-66, 2^64] and sums span e^-49..e^56.
            # s = m * 2^(e-127), m in [1,2):
            #   lse = (e - 127)*ln2 + Ln(m)
            acc = [misc.tile([128, 16], f32, name=f"acc{i}", tag="acc",
                             bufs=2) for i in range(2)]
            nc.vector.memset(acc[0][:], 0.0)
            cur = 0
            for hh in range(8):
                u = sums_t[hh][:].bitcast(i32)
                ei = stage.tile([128, 16], i32, name=f"ei{hh}", tag="ei")
                nc.vector.tensor_scalar(out=ei[:], in0=u, scalar1=23,
                                        scalar2=None, op0=SHR)
                ef = stage.tile([128, 16], f32, name=f"ef{hh}", tag="ef")
                nc.vector.tensor_copy(ef[:], ei[:])
                nc.vector.tensor_scalar(out=ef[:], in0=ef[:], scalar1=127.0,
                                        scalar2=None, op0=SUB)
                mb = stage.tile([128, 16], i32, name=f"mb{hh}", tag="mb")
                nc.vector.tensor_scalar(out=mb[:], in0=u,
                                        scalar1=0x007FFFFF,
                                        scalar2=0x3F800000,
                                        op0=AND, op1=OR)
                lnm = stage.tile([128, 16], f32, name=f"lnm{hh}", tag="lnm")
                nc.scalar.activation(lnm[:], mb[:].bitcast(f32), LN,
                                     bias=gate[:, 0:1])
                lse = stage.tile([128, 16], f32, name=f"lse{hh}", tag="lse")
                nc.vector.scalar_tensor_tensor(
                    out=lse[:], in0=ef[:], scalar=LN2, in1=lnm[:],
                    op0=MUL, op1=ADD)
                nxt = 1 - cur
                nc.vector.scalar_tensor_tensor(
                    out=acc[nxt][:], in0=lse[:], scalar=wb[:, hh:hh + 1],
                    in1=acc[cur][:], op0=MUL, op1=ADD)
                cur = nxt
            nc.sync.dma_start(out=out_d[:], in_=acc[cur][:])

    nc.compile()
    return nc


def _get_nc():
    if "nc" not in _CACHE:
        _CACHE["nc"] = _build()
    return _CACHE["nc"]


def kernel(g, W_lift, W_proj):
    from concourse.bass_utils import run_bass_kernel_spmd

    g = np.asarray(g, dtype=np.float32)
    W_lift = np.asarray(W_lift, dtype=np.float32)
    W_proj = np.asarray(W_proj, dtype=np.float32)

    nc = _get_nc()
    ident = np.eye(128, dtype=np.float32)
    tri = np.tril(np.ones((128, 128), dtype=np.float32)).astype(
        ml_dtypes.bfloat16)

    in_maps = []
    for core in range(NCORES):
        b, hg = core // 2, core % 2
        rows = []
        for p in range(PAIRS):
            h0 = hg * 8 + 2 * p
            h1 = h0 + 1
            rows += list(range(h0 * D, h0 * D + D))
            rows += list(range(h1 * D, h1 * D + D))
            rows += list(range(E + h0 * D, E + h0 * D + D))
            rows += list(range(E + h1 * D, E + h1 * D + D))
        w_slice = np.ascontiguousarray(W_lift[rows, :])
        in_maps.append({
            "g_b": np.ascontiguousarray(g[b]),
            "w_slice": w_slice,
            "wp_cols": np.ascontiguousarray(W_proj[:, hg * 8:hg * 8 + 8]),
            "ident": ident,
            "tri": tri,
        })

    res = run_bass_kernel_spmd(nc, in_maps, core_ids=list(range(NCORES)))
    _CACHE["last_results"] = res
    _CACHE["last_in_maps"] = in_maps

    out = np.zeros((B, T), dtype=np.float32)
    for core in range(NCORES):
        b = core // 2
        part = res.results[core]["out_part"]  # (128, 16)
        out[b] += part.T.reshape(-1)
    return out
